# revision 22
# baseline (speedup 1.0000x reference)
"""CARC attention processor kernel for 8 Trainium2 NeuronCores — v3.

Reference computation (B=1, L=4096, C=640, H=10, D=64):
    q/k/v = hidden @ Wq/Wk/Wv, split into 10 heads of 64
    k_cat = [k, 0.42*K_bg], v_cat = [v, 0.42*V_bg]   (key length 8192)
    out   = softmax(q k_cat^T / 8) v_cat, heads merged, @ Wo + bo

Sharding: queries split 512 per core; every core computes all 10 heads for
its queries (k/v projections replicated per core).

The ScalarE exp (41.9M elements/core at ~(N+352)/1.2 ns per ACTIVATE,
N=1024) is the hard floor (~356us); the kernel keeps ScalarE saturated:
  - all-fp16 data path, host pre-arranged inputs, descriptor-friendly DMA.
  - same-head QK pairing (q duplicated onto both partition halves via
    col-tiled projection; kT split even/odd key tiles across partition
    halves) so one [128,2,512] PSUM tile = 2 key tiles of one head ->
    exp N=1024, double-buffered in 4 banks; ctx 2 banks; proj 2 banks.
  - every non-attention PE task (q/k/v projections, softmax-denominator
    broadcast, normalization, output projection) is a queue of micro work
    items (<=2 matmul-chunks each) drained between supersteps, with
    deadline ensure()s — so the PE FIFO never parks a long fill in front
    of the next score matmul and the HAM clock gate stays warm.
  - v-projection is split by head-group (0-3 / 4-7 / 8-9) so its deadline
    spreads across pairs instead of all landing in pair 0.
  - normalization: ctx leaves PSUM immediately (one DVE copy); the
    denominator broadcast / reciprocal_approx_fast / multiply run as
    deferred items inside the next pair.
  - output projection accumulates per-pair partial products into an SBUF
    fp16 accumulator (items), so the epilogue is just the last pair's
    items + one DMA; the f16->f32 output cast happens on the host.
"""

from collections import deque

import numpy as np

import concourse.bass as bass
import concourse.mybir as mybir
import concourse.tile as tile

F32 = mybir.dt.float32
F16 = mybir.dt.float16
AF = mybir.ActivationFunctionType

B, L, C = 1, 4096, 640
H, D = 10, 64
ALPHA = 0.42
N_CORES = 8
SCALE = 1.0 / np.sqrt(D)  # 0.125
Q = L // N_CORES  # 512
NKT = L // 128  # 32 key tiles per source
NCC = C // 128  # 5 contraction chunks
VGROUPS = ((0, 4), (4, 8), (8, 10))  # v-projection head groups


def emit(nc: bass.Bass):
    n_pair = H // 2

    hT = nc.declare_dram_parameter("hT", [NCC, 128, L], F16, isOutput=False)
    hq = nc.declare_dram_parameter("hq", [NCC, 128, Q], F16, isOutput=False)
    wq = nc.declare_dram_parameter("wq", [NCC, 128, C], F16, isOutput=False)
    wk = nc.declare_dram_parameter("wk", [NCC, 128, C], F16, isOutput=False)
    wv = nc.declare_dram_parameter("wv", [NCC, 128, C], F16, isOutput=False)
    wob = nc.declare_dram_parameter("wob", [H, D + 1, C], F16, isOutput=False)
    kbgS = nc.declare_dram_parameter("kbgS", [H, 128, L // 2], F16, isOutput=False)
    vbgS = nc.declare_dram_parameter(
        "vbgS", [n_pair, 128, NKT * 130], F16, isOutput=False
    )
    out = nc.declare_dram_parameter("out", [Q, C], F16, isOutput=True)

    with tile.TileContext(nc) as tc:
        with (
            tc.tile_pool(name="singles", bufs=1) as singles,
            tc.tile_pool(name="kbgp", bufs=2) as kbgp,
            tc.tile_pool(name="vbgp", bufs=2) as vbgp,
            tc.tile_pool(name="ktp", bufs=2) as ktp,
            tc.tile_pool(name="probs", bufs=4) as probs_pool,
            tc.tile_pool(name="fin", bufs=2) as fin_pool,
            tc.tile_pool(name="ps_sc", bufs=2, space="PSUM") as ps_sc,
            tc.tile_pool(name="ps_ctx", bufs=1, space="PSUM") as ps_ctx,
            tc.tile_pool(name="ps_pj", bufs=2, space="PSUM") as ps_pj,
        ):
            # ---- persistent SBUF ----
            hT_sb = singles.tile([128, NCC, L], F16, tag="hT")
            hq_sb = singles.tile([128, NCC, Q], F16, tag="hq")
            wq_sb = singles.tile([128, NCC, C], F16, tag="wq")
            wk_sb = singles.tile([128, NCC, C], F16, tag="wk")
            wv_sb = singles.tile([128, NCC, C], F16, tag="wv")
            wob_sb = singles.tile([D + 1, H, C], F16, tag="wob")
            qdup = singles.tile([128, H, Q], F16, tag="qdup")
            v2 = singles.tile([128, NKT, H, D + 1], F16, tag="v2")
            oacc = singles.tile([128, Q // 128, C], F16, tag="oacc")
            nc.vector.memset(v2[:, :, :, D : D + 1], 1.0)
            warm = singles.tile([1, 8], F32, tag="warm")
            nc.vector.memset(warm, 0.0)
            nc.scalar.activation(warm, warm, AF.Exp, scale=1.0)

            # ---- input DMAs ordered by first-use deadline ----
            kbg_t = {}
            vbg_t = {}
            kbg_t[0] = kbgp.tile([128, 2, L // 2], F16, tag="kbg", name="kbg0")
            nc.sync.dma_start(out=kbg_t[0][:, 0, :], in_=kbgS[0, :, :])
            nc.sync.dma_start(out=hq_sb, in_=hq.rearrange("i p n -> p i n"))
            nc.sync.dma_start(out=wq_sb, in_=wq.rearrange("i p n -> p i n"))

            def stage_bg(p):
                if p not in kbg_t:
                    kbg_t[p] = kbgp.tile(
                        [128, 2, L // 2], F16, tag="kbg", name=f"kbg{p}"
                    )
                    for hi in range(2):
                        nc.sync.dma_start(
                            out=kbg_t[p][:, hi, :], in_=kbgS[2 * p + hi, :, :]
                        )
                else:
                    nc.sync.dma_start(out=kbg_t[p][:, 1, :], in_=kbgS[2 * p + 1, :, :])
                vbg_t[p] = vbgp.tile([128, NKT, 130], F16, tag="vbg", name=f"vbg{p}")
                nc.sync.dma_start(
                    out=vbg_t[p].rearrange("p t c -> p (t c)"), in_=vbgS[p]
                )

            stage_bg(0)
            nc.sync.dma_start(out=wk_sb, in_=wk.rearrange("i p n -> p i n"))
            nc.sync.dma_start(out=hT_sb, in_=hT.rearrange("i p n -> p i n"))
            nc.sync.dma_start(out=wv_sb, in_=wv.rearrange("i p n -> p i n"))
            nc.sync.dma_start(out=wob_sb, in_=wob.rearrange("h p n -> p h n"))
            stage_bg(1)

            # ---------- deferred micro work items ----------
            kT_t = {}
            psum_live = {}  # fill key -> live psum tile
            sbuf_live = {}  # ctxU / rec / ctxT tiles per pair
            work = deque()
            done = set()

            def run_item(item):
                key, fn = item
                fn()
                done.add(key)

            def drain(n=1):
                for _ in range(n):
                    if work:
                        run_item(work.popleft())

            def ensure(key):
                while key not in done:
                    assert work, f"work item {key} never queued"
                    run_item(work.popleft())

            # --- q projection: head h duplicated via col-tiled matmuls
            def q_items(h):
                def fill(i0, i1, h=h):
                    if i0 == 0:
                        psum_live["q", h] = ps_pj.tile(
                            [128, Q], F32, tag="pj", name=f"qps{h}"
                        )
                    ps = psum_live["q", h]
                    for i in range(i0, i1):
                        for par in range(2):
                            nc.tensor.matmul(
                                ps[64 * par : 64 * (par + 1), :],
                                lhsT=wq_sb[:, i, 64 * h : 64 * (h + 1)],
                                rhs=hq_sb[:, i, :],
                                start=(i == 0),
                                stop=(i == NCC - 1),
                                tile_position=(0, 64 * par),
                                skip_group_check=True,
                            )

                def copy(h=h):
                    nc.vector.tensor_copy(
                        out=qdup[:, h, :], in_=psum_live.pop(("q", h))
                    )

                return [
                    (("qf", h, 0), lambda h=h: fill(0, 3)),
                    (("qf", h, 1), lambda h=h: fill(3, NCC)),
                    (("q", h), copy),
                ]

            # --- k projection: pair p, head-in-pair hi, fill f (1024 keys,
            # even tiles -> partitions 0:64, odd -> 64:128)
            def k_items(p, hi, f):
                h = 2 * p + hi

                def chunk(i0, i1, p=p, hi=hi, f=f, h=h):
                    if i0 == 0:
                        psum_live["k", p, hi, f] = ps_pj.tile(
                            [128, Q], F32, tag="pj", name=f"kps{p}{hi}{f}"
                        )
                    ps = psum_live["k", p, hi, f]
                    hT_blk = hT_sb[:, :, 1024 * f : 1024 * (f + 1)].rearrange(
                        "p i (a b n) -> p i a b n", b=2, n=128
                    )
                    for i in range(i0, i1):
                        for par in range(2):
                            nc.tensor.matmul(
                                ps[64 * par : 64 * (par + 1), :],
                                lhsT=wk_sb[:, i, 64 * h : 64 * (h + 1)],
                                rhs=hT_blk[:, i, :, par, :],
                                start=(i == 0),
                                stop=(i == NCC - 1),
                                tile_position=(0, 64 * par),
                                skip_group_check=True,
                            )

                def copy(p=p, hi=hi, f=f):
                    nc.vector.tensor_copy(
                        out=kT_t[p][:, hi, f, :], in_=psum_live.pop(("k", p, hi, f))
                    )

                return [
                    (("kf", p, hi, f, 0), lambda p=p, hi=hi, f=f: chunk(0, 2)),
                    (("kf", p, hi, f, 1), lambda p=p, hi=hi, f=f: chunk(2, 4)),
                    (("kf", p, hi, f, 2), lambda p=p, hi=hi, f=f: chunk(4, NCC)),
                    (("k", p, hi, f), copy),
                ]

            def queue_kproj(p):
                kT_t[p] = ktp.tile([128, 2, 4, Q], F16, tag="kT", name=f"kT{p}")
                for hi in range(2):
                    for f in range(4):
                        work.extend(k_items(p, hi, f))

            # --- v projection for head group g, key tile t
            def v_items(g, t):
                h0, h1 = VGROUPS[g]
                nh = h1 - h0

                def chunk(i0, i1, g=g, t=t, h0=h0, nh=nh):
                    if i0 == 0:
                        psum_live["v", g, t] = ps_pj.tile(
                            [128, Q], F32, tag="pj", name=f"vps{g}_{t}"
                        )
                    ps = psum_live["v", g, t]
                    for i in range(i0, i1):
                        nc.tensor.matmul(
                            ps[:, 0 : 64 * nh],
                            lhsT=hT_sb[:, i, 128 * t : 128 * (t + 1)],
                            rhs=wv_sb[:, i, 64 * h0 : 64 * (h0 + nh)],
                            start=(i == 0),
                            stop=(i == NCC - 1),
                        )

                def copy(g=g, t=t, h0=h0, nh=nh):
                    ps = psum_live.pop(("v", g, t))
                    nc.vector.tensor_copy(
                        out=v2[:, t, h0 : h0 + nh, 0:D],
                        in_=ps[:, 0 : 64 * nh].rearrange("p (h d) -> p h d", d=64),
                    )

                return [
                    (("vf", g, t, 0), lambda g=g, t=t: chunk(0, 3)),
                    (("vf", g, t, 1), lambda g=g, t=t: chunk(3, NCC)),
                    (("v", g, t), copy),
                ]

            # --- denominator reciprocal items for pair p: spread the denom
            # row [1, 512] onto 128 partitions via a tiny SBUF->SBUF DMA
            # (recS[p, j] = den[128j + p]), then one cheap 4-wide reciprocal
            def n_items(p):
                items = []
                for hi in range(2):

                    def dd_fn(p=p, hi=hi):
                        ctxU = sbuf_live["ctxU", p]
                        recS = fin_pool.tile(
                            [128, Q // 128], F16, tag="dn", name=f"dn{p}{hi}", bufs=4
                        )
                        sbuf_live["dn", p, hi] = recS
                        for qt in range(Q // 128):
                            nc.sync.dma_start(
                                out=recS[:, qt : qt + 1],
                                in_=ctxU[D : D + 1, hi, 128 * qt : 128 * (qt + 1)],
                            )

                    def dr_fn(p=p, hi=hi):
                        recT = fin_pool.tile(
                            [128, Q // 128], F32, tag="dr", name=f"dr{p}{hi}", bufs=4
                        )
                        sbuf_live["dr", p, hi] = recT
                        nc.vector.reciprocal(recT, sbuf_live.pop(("dn", p, hi)))

                    items += [
                        (("dd", p, hi), dd_fn),
                        (("dr", p, hi), dr_fn),
                    ]
                return items

            # --- output projection partials for pair p: matmul against the
            # UNnormalized ctxU (denom row x wob bias row included), then a
            # per-out-partition multiply by 1/denom while accumulating.
            # (ctx@Wo + den*bias)/den = ctx_norm@Wo + bias for head 0; other
            # heads have zero bias rows.
            def o_items(p):
                items = []
                for qt in range(Q // 128):
                    for half in range(2):
                        n0 = 320 * half

                        def o_fn(p=p, qt=qt, n0=n0):
                            ctxU = sbuf_live["ctxU", p]
                            for hi in range(2):
                                ps = ps_pj.tile(
                                    [128, Q], F32, tag="pj", name=f"ops{p}{qt}{n0}{hi}"
                                )
                                nc.tensor.matmul(
                                    ps[:, 0:320],
                                    lhsT=ctxU[:, hi, 128 * qt : 128 * (qt + 1)],
                                    rhs=wob_sb[:, 2 * p + hi, n0 : n0 + 320],
                                    start=True,
                                    stop=True,
                                )
                                recT = sbuf_live["dr", p, hi]
                                if p == 0 and hi == 0:
                                    nc.vector.tensor_scalar_mul(
                                        oacc[:, qt, n0 : n0 + 320],
                                        ps[:, 0:320],
                                        recT[:, qt : qt + 1],
                                    )
                                else:
                                    nc.vector.scalar_tensor_tensor(
                                        out=oacc[:, qt, n0 : n0 + 320],
                                        in0=ps[:, 0:320],
                                        scalar=recT[:, qt : qt + 1],
                                        in1=oacc[:, qt, n0 : n0 + 320],
                                        op0=mybir.AluOpType.mult,
                                        op1=mybir.AluOpType.add,
                                    )

                        items.append((("o", p, qt, half), o_fn))
                return items

            # ---- prologue: q head 0 inline, everything else queued ----
            for it in q_items(0):
                run_item(it)
            for h in range(1, H):
                work.extend(q_items(h))
            queue_kproj(0)
            for t in range(NKT):
                work.extend(v_items(0, t))
            queue_kproj(1)
            for t in range(NKT):
                work.extend(v_items(1, t))

            # ---- attention ----
            for p in range(n_pair):
                ctx2 = ps_ctx.tile([D + 1, 2, Q], F32, tag="ctx", name=f"ctx{p}")
                if p + 2 < n_pair:
                    stage_bg(p + 2)
                if p + 1 < n_pair and p + 1 not in kT_t:
                    queue_kproj(p + 1)
                if p == 2:
                    for t in range(NKT):
                        work.extend(v_items(2, t))

                pend = deque()

                def superstep(kind, ss, hi, p=p):
                    h = 2 * p + hi
                    if kind == "bg":
                        klhs = lambda par: kbg_t[p][
                            64 * par : 64 * (par + 1), hi, 128 * ss : 128 * (ss + 1)
                        ]
                    else:
                        f, c4 = ss // 4, ss % 4
                        klhs = lambda par: kT_t[p][
                            64 * par : 64 * (par + 1), hi, f, 128 * c4 : 128 * (c4 + 1)
                        ]
                    sc = ps_sc.tile(
                        [128, 2, Q], F32, tag="sc", name=f"sc{kind}{p}{ss}{hi}"
                    )
                    # 4 concurrent 64x64 quadrant matmuls: row = even/odd key
                    # tile (kT layout), col = lo/hi key half; each gets its
                    # own XBUS stream so both tiles finish in ~512 cycles.
                    for par in range(2):
                        kl = klhs(par)
                        for co in range(2):
                            nc.tensor.matmul(
                                sc[64 * co : 64 * (co + 1), par, :],
                                lhsT=kl[:, 64 * co : 64 * (co + 1)],
                                rhs=qdup[64 * par : 64 * (par + 1), h, :],
                                start=True,
                                stop=True,
                                tile_position=(64 * par, 64 * co),
                                skip_group_check=True,
                            )
                    pr = probs_pool.tile(
                        [128, 2, Q], F16, tag="pr", name=f"pr{kind}{p}{ss}{hi}"
                    )
                    nc.scalar.activation(pr, sc, AF.Exp, scale=SCALE)
                    return pr

                def do_pv(kind, ss, hi, pr, p=p, ctx2=ctx2):
                    h = 2 * p + hi
                    g = 0 if h < 4 else (1 if h < 8 else 2)
                    for j in range(2):
                        t = 2 * ss + j
                        if kind == "bg":
                            vlhs = vbg_t[p][:, t, 65 * hi : 65 * (hi + 1)]
                        else:
                            ensure(("v", g, t))
                            vlhs = v2[:, t, h, :]
                        first = kind == "bg" and ss == 0 and j == 0
                        last = kind == "self" and ss == 15 and j == 1
                        nc.tensor.matmul(
                            ctx2[:, hi, :],
                            lhsT=vlhs,
                            rhs=pr[:, j, :],
                            start=first,
                            stop=last,
                        )

                steps = [("bg", ss, hi) for ss in range(16) for hi in range(2)]
                steps += [("self", ss, hi) for ss in range(16) for hi in range(2)]
                for si, (kind, ss, hi) in enumerate(steps):
                    if si == 32 and p > 0:
                        work.extend(o_items(p - 1))
                    if kind == "bg" and ss == 0:
                        ensure(("q", 2 * p + hi))
                    if kind == "self" and ss == 0:
                        for f in range(4):
                            ensure(("k", p, hi, f))
                    if p == 0:
                        n_dr = 0 if si < 12 else (3 if si < 32 else 2)
                    else:
                        n_dr = 2
                    pr = superstep(kind, ss, hi)
                    drain(n_dr // 2)
                    while pend:
                        do_pv(*pend.popleft())
                    drain(n_dr - n_dr // 2)
                    pend.append((kind, ss, hi, pr))
                while pend:
                    do_pv(*pend.popleft())

                # free ctx PSUM immediately; denominators run as items
                if p >= 2:
                    # ctxU reuses pair p-2's buffer; o-items of p-2 (queued
                    # into pair p-1's self phase) must have consumed it
                    ensure(("o", p - 2, Q // 128 - 1, 1))
                ctxU = fin_pool.tile(
                    [D + 1, 2, Q], F16, tag="ctxU", name=f"cu{p}", bufs=2
                )
                sbuf_live["ctxU", p] = ctxU
                nc.vector.tensor_copy(out=ctxU, in_=ctx2)
                work.extend(n_items(p))
                if p == n_pair - 1:
                    work.extend(o_items(p))

            drain(len(work))

            # ---- final output DMA (f32 cast happens on host) ----
            nc.sync.dma_start(
                out=out.rearrange("(qt p) c -> p qt c", p=128), in_=oacc
            )
    return nc


def split_waits(nc, limit=1):
    """Hoist excess sync waits onto standalone EventSemaphore instructions."""
    cnt = 0
    for f in nc.m.functions:
        for bb in f.blocks:
            fixed = []
            for inst in bb.instructions:
                si = inst.sync_info
                if si is not None and len(si.on_wait) > limit:
                    waits = list(si.on_wait)
                    extra, keep = waits[:-limit], waits[-limit:]
                    for w in extra:
                        cnt += 1
                        ev = mybir.InstEventSemaphore(
                            name=f"I-waitsplit-{cnt}", ins=[], outs=[]
                        )
                        ev.engine = inst.engine
                        ev.sync_info = mybir.SyncInfo(on_wait=[w], on_update=[])
                        nc.register_instruction(ev)
                        fixed.append(ev)
                    si.on_wait = keep
                fixed.append(inst)
            bb.instructions[:] = fixed
    return cnt


def build_bass():
    nc = bass.Bass()
    emit(nc)
    split_waits(nc)
    return nc


def make_in_maps(hidden_states, K_bg, V_bg, Wq, Wk, Wv, Wo, bo):
    f16 = np.float16
    hidden = np.asarray(hidden_states, np.float32)[0]  # [L, C]
    hT5 = np.ascontiguousarray(hidden.T.reshape(NCC, 128, L)).astype(f16)

    def chunk_w(W):
        return np.ascontiguousarray(
            np.asarray(W, np.float32).reshape(NCC, 128, C)
        ).astype(f16)

    wq5, wk5, wv5 = chunk_w(Wq), chunk_w(Wk), chunk_w(Wv)

    WoB = np.zeros((H, D + 1, C), np.float32)
    WoB[:, :D, :] = np.asarray(Wo, np.float32).reshape(H, D, C)
    WoB[0, D, :] = np.asarray(bo, np.float32)

    # bg K: [H, 128, L/2], alpha folded, even key tiles on rows 0:64
    KbgT = np.asarray(K_bg, np.float32).transpose(0, 2, 1) * ALPHA  # [H, D, L]
    kv = KbgT.reshape(H, D, NKT, 128)
    kbgS = np.empty((H, 128, L // 2), np.float32)
    kbgS[:, 0:D, :] = kv[:, :, 0::2, :].reshape(H, D, L // 2)
    kbgS[:, D:128, :] = kv[:, :, 1::2, :].reshape(H, D, L // 2)
    kbgS = np.ascontiguousarray(kbgS).astype(f16)

    # bg V: [n_pair, 128, NKT*130], alpha folded, ones baked at cols 64/129
    Vb = (np.asarray(V_bg, np.float32) * ALPHA).reshape(H, NKT, 128, D)
    vbgS = np.ones((H // 2, 128, NKT, 130), np.float32)
    for hi in range(2):
        vbgS[:, :, :, 65 * hi : 65 * hi + D] = Vb[hi::2].transpose(0, 2, 1, 3)
    vbgS = np.ascontiguousarray(vbgS.reshape(H // 2, 128, NKT * 130)).astype(f16)

    common = {
        "hT": hT5,
        "wq": wq5,
        "wk": wk5,
        "wv": wv5,
        "wob": WoB.astype(f16),
        "kbgS": kbgS,
        "vbgS": vbgS,
    }
    return [
        dict(common, hq=np.ascontiguousarray(hT5[:, :, Q * c : Q * (c + 1)]))
        for c in range(N_CORES)
    ]


_NC_CACHE = {}


def kernel(hidden_states, K_bg, V_bg, Wq, Wk, Wv, Wo, bo):
    if "nc" not in _NC_CACHE:
        _NC_CACHE["nc"] = build_bass()
    nc = _NC_CACHE["nc"]
    in_maps = make_in_maps(hidden_states, K_bg, V_bg, Wq, Wk, Wv, Wo, bo)
    from concourse import bass2jax

    results = bass2jax.run_bass_via_pjrt(nc, in_maps, n_cores=N_CORES)
    out = np.concatenate(
        [np.asarray(results[c]["out"], np.float32) for c in range(N_CORES)], axis=0
    )
    return out.reshape(B, L, C)


# revision 23
# speedup vs baseline: 1.0005x; 1.0005x over previous
"""CARC attention processor kernel for 8 Trainium2 NeuronCores — v3.

Reference computation (B=1, L=4096, C=640, H=10, D=64):
    q/k/v = hidden @ Wq/Wk/Wv, split into 10 heads of 64
    k_cat = [k, 0.42*K_bg], v_cat = [v, 0.42*V_bg]   (key length 8192)
    out   = softmax(q k_cat^T / 8) v_cat, heads merged, @ Wo + bo

Sharding: queries split 512 per core; every core computes all 10 heads for
its queries (k/v projections replicated per core).

The ScalarE exp (41.9M elements/core at ~(N+352)/1.2 ns per ACTIVATE,
N=1024) is the hard floor (~356us); the kernel keeps ScalarE saturated:
  - all-fp16 data path, host pre-arranged inputs, descriptor-friendly DMA.
  - same-head QK pairing (q duplicated onto both partition halves via
    col-tiled projection; kT split even/odd key tiles across partition
    halves) so one [128,2,512] PSUM tile = 2 key tiles of one head ->
    exp N=1024, double-buffered in 4 banks; ctx 2 banks; proj 2 banks.
  - every non-attention PE task (q/k/v projections, softmax-denominator
    broadcast, normalization, output projection) is a queue of micro work
    items (<=2 matmul-chunks each) drained between supersteps, with
    deadline ensure()s — so the PE FIFO never parks a long fill in front
    of the next score matmul and the HAM clock gate stays warm.
  - v-projection is split by head-group (0-3 / 4-7 / 8-9) so its deadline
    spreads across pairs instead of all landing in pair 0.
  - normalization: ctx leaves PSUM immediately (one DVE copy); the
    denominator broadcast / reciprocal_approx_fast / multiply run as
    deferred items inside the next pair.
  - output projection accumulates per-pair partial products into an SBUF
    fp16 accumulator (items), so the epilogue is just the last pair's
    items + one DMA; the f16->f32 output cast happens on the host.
"""

from collections import deque

import numpy as np

import concourse.bass as bass
import concourse.mybir as mybir
import concourse.tile as tile

F32 = mybir.dt.float32
F16 = mybir.dt.float16
AF = mybir.ActivationFunctionType

B, L, C = 1, 4096, 640
H, D = 10, 64
ALPHA = 0.42
N_CORES = 8
SCALE = 1.0 / np.sqrt(D)  # 0.125
Q = L // N_CORES  # 512
NKT = L // 128  # 32 key tiles per source
NCC = C // 128  # 5 contraction chunks
VGROUPS = ((0, 4), (4, 8), (8, 10))  # v-projection head groups


def emit(nc: bass.Bass):
    n_pair = H // 2

    hT = nc.declare_dram_parameter("hT", [NCC, 128, L], F16, isOutput=False)
    hq = nc.declare_dram_parameter("hq", [NCC, 128, Q], F16, isOutput=False)
    wq = nc.declare_dram_parameter("wq", [NCC, 128, C], F16, isOutput=False)
    wk = nc.declare_dram_parameter("wk", [NCC, 128, C], F16, isOutput=False)
    wv = nc.declare_dram_parameter("wv", [NCC, 128, C], F16, isOutput=False)
    wob = nc.declare_dram_parameter("wob", [H, D + 1, C], F16, isOutput=False)
    kbgS = nc.declare_dram_parameter("kbgS", [H, 128, L // 2], F16, isOutput=False)
    vbgS = nc.declare_dram_parameter(
        "vbgS", [n_pair, 128, NKT * 130], F16, isOutput=False
    )
    out = nc.declare_dram_parameter("out", [Q, C], F16, isOutput=True)

    with tile.TileContext(nc) as tc:
        with (
            tc.tile_pool(name="singles", bufs=1) as singles,
            tc.tile_pool(name="kbgp", bufs=2) as kbgp,
            tc.tile_pool(name="vbgp", bufs=2) as vbgp,
            tc.tile_pool(name="ktp", bufs=2) as ktp,
            tc.tile_pool(name="probs", bufs=3) as probs_pool,
            tc.tile_pool(name="fin", bufs=2) as fin_pool,
            tc.tile_pool(name="ps_sc", bufs=2, space="PSUM") as ps_sc,
            tc.tile_pool(name="ps_ctx", bufs=1, space="PSUM") as ps_ctx,
            tc.tile_pool(name="ps_pj", bufs=2, space="PSUM") as ps_pj,
        ):
            # ---- persistent SBUF ----
            hT_sb = singles.tile([128, NCC, L], F16, tag="hT")
            hq_sb = singles.tile([128, NCC, Q], F16, tag="hq")
            wq_sb = singles.tile([128, NCC, C], F16, tag="wq")
            wk_sb = singles.tile([128, NCC, C], F16, tag="wk")
            wv_sb = singles.tile([128, NCC, C], F16, tag="wv")
            wob_sb = singles.tile([D + 1, H, C], F16, tag="wob")
            qdup = singles.tile([128, H, Q], F16, tag="qdup")
            v2 = singles.tile([128, NKT, H, D + 1], F16, tag="v2")
            oacc = singles.tile([128, Q // 128, C], F16, tag="oacc")
            nc.vector.memset(v2[:, :, :, D : D + 1], 1.0)
            warm = singles.tile([1, 8], F32, tag="warm")
            nc.vector.memset(warm, 0.0)
            nc.scalar.activation(warm, warm, AF.Exp, scale=1.0)

            # ---- input DMAs ordered by first-use deadline ----
            kbg_t = {}
            vbg_t = {}
            kbg_t[0] = kbgp.tile([128, 2, L // 2], F16, tag="kbg", name="kbg0")
            nc.sync.dma_start(out=kbg_t[0][:, 0, :], in_=kbgS[0, :, :])
            nc.sync.dma_start(out=hq_sb, in_=hq.rearrange("i p n -> p i n"))
            nc.sync.dma_start(out=wq_sb, in_=wq.rearrange("i p n -> p i n"))

            def stage_bg(p):
                if p not in kbg_t:
                    kbg_t[p] = kbgp.tile(
                        [128, 2, L // 2], F16, tag="kbg", name=f"kbg{p}"
                    )
                    for hi in range(2):
                        nc.sync.dma_start(
                            out=kbg_t[p][:, hi, :], in_=kbgS[2 * p + hi, :, :]
                        )
                else:
                    nc.sync.dma_start(out=kbg_t[p][:, 1, :], in_=kbgS[2 * p + 1, :, :])
                vbg_t[p] = vbgp.tile([128, NKT, 130], F16, tag="vbg", name=f"vbg{p}")
                nc.sync.dma_start(
                    out=vbg_t[p].rearrange("p t c -> p (t c)"), in_=vbgS[p]
                )

            stage_bg(0)
            nc.sync.dma_start(out=wk_sb, in_=wk.rearrange("i p n -> p i n"))
            nc.sync.dma_start(out=hT_sb, in_=hT.rearrange("i p n -> p i n"))
            nc.sync.dma_start(out=wv_sb, in_=wv.rearrange("i p n -> p i n"))
            nc.sync.dma_start(out=wob_sb, in_=wob.rearrange("h p n -> p h n"))
            stage_bg(1)

            # ---------- deferred micro work items ----------
            kT_t = {}
            psum_live = {}  # fill key -> live psum tile
            sbuf_live = {}  # ctxU / rec / ctxT tiles per pair
            work = deque()
            done = set()

            def run_item(item):
                key, fn = item
                fn()
                done.add(key)

            def drain(n=1):
                for _ in range(n):
                    if work:
                        run_item(work.popleft())

            def ensure(key):
                while key not in done:
                    assert work, f"work item {key} never queued"
                    run_item(work.popleft())

            # --- q projection: head h duplicated via col-tiled matmuls
            def q_items(h):
                def fill(i0, i1, h=h):
                    if i0 == 0:
                        psum_live["q", h] = ps_pj.tile(
                            [128, Q], F32, tag="pj", name=f"qps{h}"
                        )
                    ps = psum_live["q", h]
                    for i in range(i0, i1):
                        for par in range(2):
                            nc.tensor.matmul(
                                ps[64 * par : 64 * (par + 1), :],
                                lhsT=wq_sb[:, i, 64 * h : 64 * (h + 1)],
                                rhs=hq_sb[:, i, :],
                                start=(i == 0),
                                stop=(i == NCC - 1),
                                tile_position=(0, 64 * par),
                                skip_group_check=True,
                            )

                def copy(h=h):
                    nc.vector.tensor_copy(
                        out=qdup[:, h, :], in_=psum_live.pop(("q", h))
                    )

                return [
                    (("qf", h, 0), lambda h=h: fill(0, 3)),
                    (("qf", h, 1), lambda h=h: fill(3, NCC)),
                    (("q", h), copy),
                ]

            # --- k projection: pair p, head-in-pair hi, fill f (1024 keys,
            # even tiles -> partitions 0:64, odd -> 64:128)
            def k_items(p, hi, f):
                h = 2 * p + hi

                def chunk(i0, i1, p=p, hi=hi, f=f, h=h):
                    if i0 == 0:
                        psum_live["k", p, hi, f] = ps_pj.tile(
                            [128, Q], F32, tag="pj", name=f"kps{p}{hi}{f}"
                        )
                    ps = psum_live["k", p, hi, f]
                    hT_blk = hT_sb[:, :, 1024 * f : 1024 * (f + 1)].rearrange(
                        "p i (a b n) -> p i a b n", b=2, n=128
                    )
                    for i in range(i0, i1):
                        for par in range(2):
                            nc.tensor.matmul(
                                ps[64 * par : 64 * (par + 1), :],
                                lhsT=wk_sb[:, i, 64 * h : 64 * (h + 1)],
                                rhs=hT_blk[:, i, :, par, :],
                                start=(i == 0),
                                stop=(i == NCC - 1),
                                tile_position=(0, 64 * par),
                                skip_group_check=True,
                            )

                def copy(p=p, hi=hi, f=f):
                    nc.vector.tensor_copy(
                        out=kT_t[p][:, hi, f, :], in_=psum_live.pop(("k", p, hi, f))
                    )

                return [
                    (("kf", p, hi, f, 0), lambda p=p, hi=hi, f=f: chunk(0, 2)),
                    (("kf", p, hi, f, 1), lambda p=p, hi=hi, f=f: chunk(2, 4)),
                    (("kf", p, hi, f, 2), lambda p=p, hi=hi, f=f: chunk(4, NCC)),
                    (("k", p, hi, f), copy),
                ]

            def queue_kproj(p):
                kT_t[p] = ktp.tile([128, 2, 4, Q], F16, tag="kT", name=f"kT{p}")
                for hi in range(2):
                    for f in range(4):
                        work.extend(k_items(p, hi, f))

            # --- v projection for head group g, key tile t
            def v_items(g, t):
                h0, h1 = VGROUPS[g]
                nh = h1 - h0

                def chunk(i0, i1, g=g, t=t, h0=h0, nh=nh):
                    if i0 == 0:
                        psum_live["v", g, t] = ps_pj.tile(
                            [128, Q], F32, tag="pj", name=f"vps{g}_{t}"
                        )
                    ps = psum_live["v", g, t]
                    for i in range(i0, i1):
                        nc.tensor.matmul(
                            ps[:, 0 : 64 * nh],
                            lhsT=hT_sb[:, i, 128 * t : 128 * (t + 1)],
                            rhs=wv_sb[:, i, 64 * h0 : 64 * (h0 + nh)],
                            start=(i == 0),
                            stop=(i == NCC - 1),
                        )

                def copy(g=g, t=t, h0=h0, nh=nh):
                    ps = psum_live.pop(("v", g, t))
                    nc.vector.tensor_copy(
                        out=v2[:, t, h0 : h0 + nh, 0:D],
                        in_=ps[:, 0 : 64 * nh].rearrange("p (h d) -> p h d", d=64),
                    )

                return [
                    (("vf", g, t, 0), lambda g=g, t=t: chunk(0, 3)),
                    (("vf", g, t, 1), lambda g=g, t=t: chunk(3, NCC)),
                    (("v", g, t), copy),
                ]

            # --- denominator reciprocal items for pair p: spread the denom
            # row [1, 512] onto 128 partitions via a tiny SBUF->SBUF DMA
            # (recS[p, j] = den[128j + p]), then one cheap 4-wide reciprocal
            def n_items(p):
                items = []
                for hi in range(2):

                    def dd_fn(p=p, hi=hi):
                        ctxU = sbuf_live["ctxU", p]
                        recS = fin_pool.tile(
                            [128, Q // 128], F16, tag="dn", name=f"dn{p}{hi}", bufs=4
                        )
                        sbuf_live["dn", p, hi] = recS
                        for qt in range(Q // 128):
                            nc.sync.dma_start(
                                out=recS[:, qt : qt + 1],
                                in_=ctxU[D : D + 1, hi, 128 * qt : 128 * (qt + 1)],
                            )

                    def dr_fn(p=p, hi=hi):
                        recT = fin_pool.tile(
                            [128, Q // 128], F32, tag="dr", name=f"dr{p}{hi}", bufs=4
                        )
                        sbuf_live["dr", p, hi] = recT
                        nc.vector.reciprocal(recT, sbuf_live.pop(("dn", p, hi)))

                    items += [
                        (("dd", p, hi), dd_fn),
                        (("dr", p, hi), dr_fn),
                    ]
                return items

            # --- output projection partials for pair p: matmul against the
            # UNnormalized ctxU (denom row x wob bias row included), then a
            # per-out-partition multiply by 1/denom while accumulating.
            # (ctx@Wo + den*bias)/den = ctx_norm@Wo + bias for head 0; other
            # heads have zero bias rows.
            def o_items(p):
                items = []
                for qt in range(Q // 128):
                    for half in range(2):
                        n0 = 320 * half

                        def o_fn(p=p, qt=qt, n0=n0):
                            ctxU = sbuf_live["ctxU", p]
                            for hi in range(2):
                                ps = ps_pj.tile(
                                    [128, Q], F32, tag="pj", name=f"ops{p}{qt}{n0}{hi}"
                                )
                                nc.tensor.matmul(
                                    ps[:, 0:320],
                                    lhsT=ctxU[:, hi, 128 * qt : 128 * (qt + 1)],
                                    rhs=wob_sb[:, 2 * p + hi, n0 : n0 + 320],
                                    start=True,
                                    stop=True,
                                )
                                recT = sbuf_live["dr", p, hi]
                                if p == 0 and hi == 0:
                                    nc.vector.tensor_scalar_mul(
                                        oacc[:, qt, n0 : n0 + 320],
                                        ps[:, 0:320],
                                        recT[:, qt : qt + 1],
                                    )
                                else:
                                    nc.vector.scalar_tensor_tensor(
                                        out=oacc[:, qt, n0 : n0 + 320],
                                        in0=ps[:, 0:320],
                                        scalar=recT[:, qt : qt + 1],
                                        in1=oacc[:, qt, n0 : n0 + 320],
                                        op0=mybir.AluOpType.mult,
                                        op1=mybir.AluOpType.add,
                                    )

                        items.append((("o", p, qt, half), o_fn))
                return items

            # ---- prologue: q head 0 inline, everything else queued ----
            for it in q_items(0):
                run_item(it)
            for h in range(1, H):
                work.extend(q_items(h))
            queue_kproj(0)
            for t in range(NKT):
                work.extend(v_items(0, t))
            queue_kproj(1)
            for t in range(NKT):
                work.extend(v_items(1, t))

            # ---- attention ----
            for p in range(n_pair):
                ctx2 = ps_ctx.tile([D + 1, 2, Q], F32, tag="ctx", name=f"ctx{p}")
                if p + 2 < n_pair:
                    stage_bg(p + 2)
                if p + 1 < n_pair and p + 1 not in kT_t:
                    queue_kproj(p + 1)
                if p == 2:
                    for t in range(NKT):
                        work.extend(v_items(2, t))

                pend = deque()

                def superstep(kind, ss, hi, p=p):
                    h = 2 * p + hi
                    if kind == "bg":
                        klhs = lambda par: kbg_t[p][
                            64 * par : 64 * (par + 1), hi, 128 * ss : 128 * (ss + 1)
                        ]
                    else:
                        f, c4 = ss // 4, ss % 4
                        klhs = lambda par: kT_t[p][
                            64 * par : 64 * (par + 1), hi, f, 128 * c4 : 128 * (c4 + 1)
                        ]
                    sc = ps_sc.tile(
                        [128, 2, Q], F32, tag="sc", name=f"sc{kind}{p}{ss}{hi}"
                    )
                    # 4 concurrent 64x64 quadrant matmuls: row = even/odd key
                    # tile (kT layout), col = lo/hi key half; each gets its
                    # own XBUS stream so both tiles finish in ~512 cycles.
                    for par in range(2):
                        kl = klhs(par)
                        for co in range(2):
                            nc.tensor.matmul(
                                sc[64 * co : 64 * (co + 1), par, :],
                                lhsT=kl[:, 64 * co : 64 * (co + 1)],
                                rhs=qdup[64 * par : 64 * (par + 1), h, :],
                                start=True,
                                stop=True,
                                tile_position=(64 * par, 64 * co),
                                skip_group_check=True,
                            )
                    pr = probs_pool.tile(
                        [128, 2, Q], F16, tag="pr", name=f"pr{kind}{p}{ss}{hi}"
                    )
                    nc.scalar.activation(pr, sc, AF.Exp, scale=SCALE)
                    return pr

                def do_pv(kind, ss, hi, pr, p=p, ctx2=ctx2):
                    h = 2 * p + hi
                    g = 0 if h < 4 else (1 if h < 8 else 2)
                    for j in range(2):
                        t = 2 * ss + j
                        if kind == "bg":
                            vlhs = vbg_t[p][:, t, 65 * hi : 65 * (hi + 1)]
                        else:
                            ensure(("v", g, t))
                            vlhs = v2[:, t, h, :]
                        first = kind == "bg" and ss == 0 and j == 0
                        last = kind == "self" and ss == 15 and j == 1
                        nc.tensor.matmul(
                            ctx2[:, hi, :],
                            lhsT=vlhs,
                            rhs=pr[:, j, :],
                            start=first,
                            stop=last,
                        )

                steps = [("bg", ss, hi) for ss in range(16) for hi in range(2)]
                steps += [("self", ss, hi) for ss in range(16) for hi in range(2)]
                for si, (kind, ss, hi) in enumerate(steps):
                    if si == 8 and p > 0:
                        work.extend(o_items(p - 1))
                    if kind == "bg" and ss == 0:
                        ensure(("q", 2 * p + hi))
                    if kind == "self" and ss == 0:
                        for f in range(4):
                            ensure(("k", p, hi, f))
                    if p == 0:
                        n_dr = 0 if si < 12 else (3 if si < 32 else 2)
                    else:
                        n_dr = 2
                    pr = superstep(kind, ss, hi)
                    drain(n_dr // 2)
                    while pend:
                        do_pv(*pend.popleft())
                    drain(n_dr - n_dr // 2)
                    pend.append((kind, ss, hi, pr))
                while pend:
                    do_pv(*pend.popleft())

                # free ctx PSUM immediately; denominators run as items
                if p >= 2:
                    # ctxU reuses pair p-2's buffer; o-items of p-2 (queued
                    # into pair p-1's self phase) must have consumed it
                    ensure(("o", p - 2, Q // 128 - 1, 1))
                ctxU = fin_pool.tile(
                    [D + 1, 2, Q], F16, tag="ctxU", name=f"cu{p}", bufs=2
                )
                sbuf_live["ctxU", p] = ctxU
                nc.vector.tensor_copy(out=ctxU, in_=ctx2)
                work.extend(n_items(p))
                if p == n_pair - 1:
                    work.extend(o_items(p))

            drain(len(work))

            # ---- final output DMA (f32 cast happens on host) ----
            nc.sync.dma_start(
                out=out.rearrange("(qt p) c -> p qt c", p=128), in_=oacc
            )
    return nc


def split_waits(nc, limit=1):
    """Hoist excess sync waits onto standalone EventSemaphore instructions."""
    cnt = 0
    for f in nc.m.functions:
        for bb in f.blocks:
            fixed = []
            for inst in bb.instructions:
                si = inst.sync_info
                if si is not None and len(si.on_wait) > limit:
                    waits = list(si.on_wait)
                    extra, keep = waits[:-limit], waits[-limit:]
                    for w in extra:
                        cnt += 1
                        ev = mybir.InstEventSemaphore(
                            name=f"I-waitsplit-{cnt}", ins=[], outs=[]
                        )
                        ev.engine = inst.engine
                        ev.sync_info = mybir.SyncInfo(on_wait=[w], on_update=[])
                        nc.register_instruction(ev)
                        fixed.append(ev)
                    si.on_wait = keep
                fixed.append(inst)
            bb.instructions[:] = fixed
    return cnt


def build_bass():
    nc = bass.Bass()
    emit(nc)
    split_waits(nc)
    return nc


def make_in_maps(hidden_states, K_bg, V_bg, Wq, Wk, Wv, Wo, bo):
    f16 = np.float16
    hidden = np.asarray(hidden_states, np.float32)[0]  # [L, C]
    hT5 = np.ascontiguousarray(hidden.T.reshape(NCC, 128, L)).astype(f16)

    def chunk_w(W):
        return np.ascontiguousarray(
            np.asarray(W, np.float32).reshape(NCC, 128, C)
        ).astype(f16)

    wq5, wk5, wv5 = chunk_w(Wq), chunk_w(Wk), chunk_w(Wv)

    WoB = np.zeros((H, D + 1, C), np.float32)
    WoB[:, :D, :] = np.asarray(Wo, np.float32).reshape(H, D, C)
    WoB[0, D, :] = np.asarray(bo, np.float32)

    # bg K: [H, 128, L/2], alpha folded, even key tiles on rows 0:64
    KbgT = np.asarray(K_bg, np.float32).transpose(0, 2, 1) * ALPHA  # [H, D, L]
    kv = KbgT.reshape(H, D, NKT, 128)
    kbgS = np.empty((H, 128, L // 2), np.float32)
    kbgS[:, 0:D, :] = kv[:, :, 0::2, :].reshape(H, D, L // 2)
    kbgS[:, D:128, :] = kv[:, :, 1::2, :].reshape(H, D, L // 2)
    kbgS = np.ascontiguousarray(kbgS).astype(f16)

    # bg V: [n_pair, 128, NKT*130], alpha folded, ones baked at cols 64/129
    Vb = (np.asarray(V_bg, np.float32) * ALPHA).reshape(H, NKT, 128, D)
    vbgS = np.ones((H // 2, 128, NKT, 130), np.float32)
    for hi in range(2):
        vbgS[:, :, :, 65 * hi : 65 * hi + D] = Vb[hi::2].transpose(0, 2, 1, 3)
    vbgS = np.ascontiguousarray(vbgS.reshape(H // 2, 128, NKT * 130)).astype(f16)

    common = {
        "hT": hT5,
        "wq": wq5,
        "wk": wk5,
        "wv": wv5,
        "wob": WoB.astype(f16),
        "kbgS": kbgS,
        "vbgS": vbgS,
    }
    return [
        dict(common, hq=np.ascontiguousarray(hT5[:, :, Q * c : Q * (c + 1)]))
        for c in range(N_CORES)
    ]


_NC_CACHE = {}


def kernel(hidden_states, K_bg, V_bg, Wq, Wk, Wv, Wo, bo):
    if "nc" not in _NC_CACHE:
        _NC_CACHE["nc"] = build_bass()
    nc = _NC_CACHE["nc"]
    in_maps = make_in_maps(hidden_states, K_bg, V_bg, Wq, Wk, Wv, Wo, bo)
    from concourse import bass2jax

    results = bass2jax.run_bass_via_pjrt(nc, in_maps, n_cores=N_CORES)
    out = np.concatenate(
        [np.asarray(results[c]["out"], np.float32) for c in range(N_CORES)], axis=0
    )
    return out.reshape(B, L, C)


# revision 24
# speedup vs baseline: 1.0011x; 1.0007x over previous
"""CARC attention processor kernel for 8 Trainium2 NeuronCores — v3.

Reference computation (B=1, L=4096, C=640, H=10, D=64):
    q/k/v = hidden @ Wq/Wk/Wv, split into 10 heads of 64
    k_cat = [k, 0.42*K_bg], v_cat = [v, 0.42*V_bg]   (key length 8192)
    out   = softmax(q k_cat^T / 8) v_cat, heads merged, @ Wo + bo

Sharding: queries split 512 per core; every core computes all 10 heads for
its queries (k/v projections replicated per core).

The ScalarE exp (41.9M elements/core at ~(N+352)/1.2 ns per ACTIVATE,
N=1024) is the hard floor (~356us); the kernel keeps ScalarE saturated:
  - all-fp16 data path, host pre-arranged inputs, descriptor-friendly DMA.
  - same-head QK pairing (q duplicated onto both partition halves via
    col-tiled projection; kT split even/odd key tiles across partition
    halves) so one [128,2,512] PSUM tile = 2 key tiles of one head ->
    exp N=1024, double-buffered in 4 banks; ctx 2 banks; proj 2 banks.
  - every non-attention PE task (q/k/v projections, softmax-denominator
    broadcast, normalization, output projection) is a queue of micro work
    items (<=2 matmul-chunks each) drained between supersteps, with
    deadline ensure()s — so the PE FIFO never parks a long fill in front
    of the next score matmul and the HAM clock gate stays warm.
  - v-projection is split by head-group (0-3 / 4-7 / 8-9) so its deadline
    spreads across pairs instead of all landing in pair 0.
  - normalization: ctx leaves PSUM immediately (one DVE copy); the
    denominator broadcast / reciprocal_approx_fast / multiply run as
    deferred items inside the next pair.
  - output projection accumulates per-pair partial products into an SBUF
    fp16 accumulator (items), so the epilogue is just the last pair's
    items + one DMA; the f16->f32 output cast happens on the host.
"""

from collections import deque

import numpy as np

import concourse.bass as bass
import concourse.mybir as mybir
import concourse.tile as tile

F32 = mybir.dt.float32
F16 = mybir.dt.float16
AF = mybir.ActivationFunctionType

B, L, C = 1, 4096, 640
H, D = 10, 64
ALPHA = 0.42
N_CORES = 8
SCALE = 1.0 / np.sqrt(D)  # 0.125
Q = L // N_CORES  # 512
NKT = L // 128  # 32 key tiles per source
NCC = C // 128  # 5 contraction chunks
VGROUPS = ((0, 4), (4, 8), (8, 10))  # v-projection head groups


def emit(nc: bass.Bass):
    n_pair = H // 2

    hT = nc.declare_dram_parameter("hT", [NCC, 128, L], F16, isOutput=False)
    hq = nc.declare_dram_parameter("hq", [NCC, 128, Q], F16, isOutput=False)
    wq = nc.declare_dram_parameter("wq", [NCC, 128, C], F16, isOutput=False)
    wk = nc.declare_dram_parameter("wk", [NCC, 128, C], F16, isOutput=False)
    wv = nc.declare_dram_parameter("wv", [NCC, 128, C], F16, isOutput=False)
    wob = nc.declare_dram_parameter("wob", [H, D + 1, C], F16, isOutput=False)
    kbgS = nc.declare_dram_parameter("kbgS", [H, 128, L // 2], F16, isOutput=False)
    vbgS = nc.declare_dram_parameter(
        "vbgS", [n_pair, 128, NKT * 130], F16, isOutput=False
    )
    out = nc.declare_dram_parameter("out", [Q, C], F16, isOutput=True)

    with tile.TileContext(nc) as tc:
        with (
            tc.tile_pool(name="singles", bufs=1) as singles,
            tc.tile_pool(name="kbgp", bufs=2) as kbgp,
            tc.tile_pool(name="vbgp", bufs=2) as vbgp,
            tc.tile_pool(name="ktp", bufs=2) as ktp,
            tc.tile_pool(name="probs", bufs=3) as probs_pool,
            tc.tile_pool(name="fin", bufs=2) as fin_pool,
            tc.tile_pool(name="ps_sc", bufs=2, space="PSUM") as ps_sc,
            tc.tile_pool(name="ps_ctx", bufs=1, space="PSUM") as ps_ctx,
            tc.tile_pool(name="ps_pj", bufs=2, space="PSUM") as ps_pj,
        ):
            # ---- persistent SBUF ----
            hT_sb = singles.tile([128, NCC, L], F16, tag="hT")
            hq_sb = singles.tile([128, NCC, Q], F16, tag="hq")
            wq_sb = singles.tile([128, NCC, C], F16, tag="wq")
            wk_sb = singles.tile([128, NCC, C], F16, tag="wk")
            wv_sb = singles.tile([128, NCC, C], F16, tag="wv")
            wob_sb = singles.tile([D + 1, H, C], F16, tag="wob")
            qdup = singles.tile([128, H, Q], F16, tag="qdup")
            v2 = singles.tile([128, NKT, H, D + 1], F16, tag="v2")
            oacc = singles.tile([128, Q // 128, C], F16, tag="oacc")
            nc.vector.memset(v2[:, :, :, D : D + 1], 1.0)
            warm = singles.tile([1, 8], F32, tag="warm")
            nc.vector.memset(warm, 0.0)
            nc.scalar.activation(warm, warm, AF.Exp, scale=1.0)

            # ---- input DMAs ordered by first-use deadline ----
            kbg_t = {}
            vbg_t = {}
            kbg_t[0] = kbgp.tile([128, 2, L // 2], F16, tag="kbg", name="kbg0")
            nc.sync.dma_start(out=kbg_t[0][:, 0, :], in_=kbgS[0, :, :])
            nc.sync.dma_start(out=hq_sb, in_=hq.rearrange("i p n -> p i n"))
            nc.sync.dma_start(out=wq_sb, in_=wq.rearrange("i p n -> p i n"))

            def stage_bg(p):
                if p not in kbg_t:
                    kbg_t[p] = kbgp.tile(
                        [128, 2, L // 2], F16, tag="kbg", name=f"kbg{p}"
                    )
                    for hi in range(2):
                        nc.sync.dma_start(
                            out=kbg_t[p][:, hi, :], in_=kbgS[2 * p + hi, :, :]
                        )
                else:
                    nc.sync.dma_start(out=kbg_t[p][:, 1, :], in_=kbgS[2 * p + 1, :, :])
                vbg_t[p] = vbgp.tile([128, NKT, 130], F16, tag="vbg", name=f"vbg{p}")
                nc.sync.dma_start(
                    out=vbg_t[p].rearrange("p t c -> p (t c)"), in_=vbgS[p]
                )

            stage_bg(0)
            nc.sync.dma_start(out=wk_sb, in_=wk.rearrange("i p n -> p i n"))
            nc.sync.dma_start(out=hT_sb, in_=hT.rearrange("i p n -> p i n"))
            nc.sync.dma_start(out=wv_sb, in_=wv.rearrange("i p n -> p i n"))
            nc.sync.dma_start(out=wob_sb, in_=wob.rearrange("h p n -> p h n"))
            stage_bg(1)

            # ---------- deferred micro work items ----------
            kT_t = {}
            psum_live = {}  # fill key -> live psum tile
            sbuf_live = {}  # ctxU / rec / ctxT tiles per pair
            work = deque()
            done = set()

            def run_item(item):
                key, fn = item
                fn()
                done.add(key)

            def drain(n=1):
                for _ in range(n):
                    if work:
                        run_item(work.popleft())

            def ensure(key):
                while key not in done:
                    assert work, f"work item {key} never queued"
                    run_item(work.popleft())

            # --- q projection: head h duplicated via col-tiled matmuls
            def q_items(h):
                def fill(i0, i1, h=h):
                    if i0 == 0:
                        psum_live["q", h] = ps_pj.tile(
                            [128, Q], F32, tag="pj", name=f"qps{h}"
                        )
                    ps = psum_live["q", h]
                    for i in range(i0, i1):
                        for par in range(2):
                            nc.tensor.matmul(
                                ps[64 * par : 64 * (par + 1), :],
                                lhsT=wq_sb[:, i, 64 * h : 64 * (h + 1)],
                                rhs=hq_sb[:, i, :],
                                start=(i == 0),
                                stop=(i == NCC - 1),
                                tile_position=(0, 64 * par),
                                skip_group_check=True,
                            )

                def copy(h=h):
                    nc.vector.tensor_copy(
                        out=qdup[:, h, :], in_=psum_live.pop(("q", h))
                    )

                return [
                    (("qf", h, 0), lambda h=h: fill(0, 3)),
                    (("qf", h, 1), lambda h=h: fill(3, NCC)),
                    (("q", h), copy),
                ]

            # --- k projection: pair p, head-in-pair hi, fill f (1024 keys,
            # even tiles -> partitions 0:64, odd -> 64:128)
            def k_items(p, hi, f):
                h = 2 * p + hi

                def chunk(i0, i1, p=p, hi=hi, f=f, h=h):
                    if i0 == 0:
                        psum_live["k", p, hi, f] = ps_pj.tile(
                            [128, Q], F32, tag="pj", name=f"kps{p}{hi}{f}"
                        )
                    ps = psum_live["k", p, hi, f]
                    hT_blk = hT_sb[:, :, 1024 * f : 1024 * (f + 1)].rearrange(
                        "p i (a b n) -> p i a b n", b=2, n=128
                    )
                    for i in range(i0, i1):
                        for par in range(2):
                            nc.tensor.matmul(
                                ps[64 * par : 64 * (par + 1), :],
                                lhsT=wk_sb[:, i, 64 * h : 64 * (h + 1)],
                                rhs=hT_blk[:, i, :, par, :],
                                start=(i == 0),
                                stop=(i == NCC - 1),
                                tile_position=(0, 64 * par),
                                skip_group_check=True,
                            )

                def copy(p=p, hi=hi, f=f):
                    nc.vector.tensor_copy(
                        out=kT_t[p][:, hi, f, :], in_=psum_live.pop(("k", p, hi, f))
                    )

                return [
                    (("kf", p, hi, f, 0), lambda p=p, hi=hi, f=f: chunk(0, 2)),
                    (("kf", p, hi, f, 1), lambda p=p, hi=hi, f=f: chunk(2, 4)),
                    (("kf", p, hi, f, 2), lambda p=p, hi=hi, f=f: chunk(4, NCC)),
                    (("k", p, hi, f), copy),
                ]

            def queue_kproj(p):
                kT_t[p] = ktp.tile([128, 2, 4, Q], F16, tag="kT", name=f"kT{p}")
                for hi in range(2):
                    for f in range(4):
                        work.extend(k_items(p, hi, f))

            # --- v projection for head group g, key tile t
            def v_items(g, t):
                h0, h1 = VGROUPS[g]
                nh = h1 - h0

                def chunk(i0, i1, g=g, t=t, h0=h0, nh=nh):
                    if i0 == 0:
                        psum_live["v", g, t] = ps_pj.tile(
                            [128, Q], F32, tag="pj", name=f"vps{g}_{t}"
                        )
                    ps = psum_live["v", g, t]
                    for i in range(i0, i1):
                        nc.tensor.matmul(
                            ps[:, 0 : 64 * nh],
                            lhsT=hT_sb[:, i, 128 * t : 128 * (t + 1)],
                            rhs=wv_sb[:, i, 64 * h0 : 64 * (h0 + nh)],
                            start=(i == 0),
                            stop=(i == NCC - 1),
                        )

                def copy(g=g, t=t, h0=h0, nh=nh):
                    ps = psum_live.pop(("v", g, t))
                    nc.vector.tensor_copy(
                        out=v2[:, t, h0 : h0 + nh, 0:D],
                        in_=ps[:, 0 : 64 * nh].rearrange("p (h d) -> p h d", d=64),
                    )

                return [
                    (("vf", g, t, 0), lambda g=g, t=t: chunk(0, 3)),
                    (("vf", g, t, 1), lambda g=g, t=t: chunk(3, NCC)),
                    (("v", g, t), copy),
                ]

            # --- denominator reciprocal items for pair p: spread the denom
            # row [1, 512] onto 128 partitions via a tiny SBUF->SBUF DMA
            # (recS[p, j] = den[128j + p]), then one cheap 4-wide reciprocal
            def n_items(p):
                items = []
                for hi in range(2):

                    def dd_fn(p=p, hi=hi):
                        ctxU = sbuf_live["ctxU", p]
                        recS = fin_pool.tile(
                            [128, Q // 128], F16, tag="dn", name=f"dn{p}{hi}", bufs=4
                        )
                        sbuf_live["dn", p, hi] = recS
                        for qt in range(Q // 128):
                            nc.sync.dma_start(
                                out=recS[:, qt : qt + 1],
                                in_=ctxU[D : D + 1, hi, 128 * qt : 128 * (qt + 1)],
                            )

                    def dr_fn(p=p, hi=hi):
                        recT = fin_pool.tile(
                            [128, Q // 128], F32, tag="dr", name=f"dr{p}{hi}", bufs=4
                        )
                        sbuf_live["dr", p, hi] = recT
                        nc.vector.reciprocal(recT, sbuf_live.pop(("dn", p, hi)))

                    items += [
                        (("dd", p, hi), dd_fn),
                        (("dr", p, hi), dr_fn),
                    ]
                return items

            # --- output projection partials for pair p: matmul against the
            # UNnormalized ctxU (denom row x wob bias row included), then a
            # per-out-partition multiply by 1/denom while accumulating.
            # (ctx@Wo + den*bias)/den = ctx_norm@Wo + bias for head 0; other
            # heads have zero bias rows.
            def o_items(p):
                items = []
                for qt in range(Q // 128):
                    for half in range(2):
                        n0 = 320 * half

                        def o_fn(p=p, qt=qt, half=half, n0=n0):
                            ctxU = sbuf_live["ctxU", p]
                            for hi in range(2):
                                ps = ps_pj.tile(
                                    [128, Q], F32, tag="pj", name=f"ops{p}{qt}{n0}{hi}"
                                )
                                nc.tensor.matmul(
                                    ps[:, 0:320],
                                    lhsT=ctxU[:, hi, 128 * qt : 128 * (qt + 1)],
                                    rhs=wob_sb[:, 2 * p + hi, n0 : n0 + 320],
                                    start=True,
                                    stop=True,
                                )
                                recT = sbuf_live["dr", p, hi]
                                if p == 0 and hi == 0:
                                    nc.vector.tensor_scalar_mul(
                                        oacc[:, qt, n0 : n0 + 320],
                                        ps[:, 0:320],
                                        recT[:, qt : qt + 1],
                                    )
                                else:
                                    nc.vector.scalar_tensor_tensor(
                                        out=oacc[:, qt, n0 : n0 + 320],
                                        in0=ps[:, 0:320],
                                        scalar=recT[:, qt : qt + 1],
                                        in1=oacc[:, qt, n0 : n0 + 320],
                                        op0=mybir.AluOpType.mult,
                                        op1=mybir.AluOpType.add,
                                    )
                            if p == n_pair - 1 and half == 1:
                                o_dma(qt)

                        items.append((("o", p, qt, half), o_fn))
                return items

            def o_dma(qt):
                nc.sync.dma_start(
                    out=out[128 * qt : 128 * (qt + 1), :], in_=oacc[:, qt, :]
                )

            # ---- prologue: q head 0 inline, everything else queued ----
            for it in q_items(0):
                run_item(it)
            for h in range(1, H):
                work.extend(q_items(h))
            queue_kproj(0)
            for t in range(NKT):
                work.extend(v_items(0, t))
            queue_kproj(1)
            for t in range(NKT):
                work.extend(v_items(1, t))

            # ---- attention ----
            for p in range(n_pair):
                ctx2 = ps_ctx.tile([D + 1, 2, Q], F32, tag="ctx", name=f"ctx{p}")
                if p + 2 < n_pair:
                    stage_bg(p + 2)
                if p + 1 < n_pair and p + 1 not in kT_t:
                    queue_kproj(p + 1)
                if p == 2:
                    for t in range(NKT):
                        work.extend(v_items(2, t))

                pend = deque()

                def superstep(kind, ss, hi, p=p):
                    h = 2 * p + hi
                    if kind == "bg":
                        klhs = lambda par: kbg_t[p][
                            64 * par : 64 * (par + 1), hi, 128 * ss : 128 * (ss + 1)
                        ]
                    else:
                        f, c4 = ss // 4, ss % 4
                        klhs = lambda par: kT_t[p][
                            64 * par : 64 * (par + 1), hi, f, 128 * c4 : 128 * (c4 + 1)
                        ]
                    sc = ps_sc.tile(
                        [128, 2, Q], F32, tag="sc", name=f"sc{kind}{p}{ss}{hi}"
                    )
                    # 4 concurrent 64x64 quadrant matmuls: row = even/odd key
                    # tile (kT layout), col = lo/hi key half; each gets its
                    # own XBUS stream so both tiles finish in ~512 cycles.
                    for par in range(2):
                        kl = klhs(par)
                        for co in range(2):
                            nc.tensor.matmul(
                                sc[64 * co : 64 * (co + 1), par, :],
                                lhsT=kl[:, 64 * co : 64 * (co + 1)],
                                rhs=qdup[64 * par : 64 * (par + 1), h, :],
                                start=True,
                                stop=True,
                                tile_position=(64 * par, 64 * co),
                                skip_group_check=True,
                            )
                    pr = probs_pool.tile(
                        [128, 2, Q], F16, tag="pr", name=f"pr{kind}{p}{ss}{hi}"
                    )
                    nc.scalar.activation(pr, sc, AF.Exp, scale=SCALE)
                    return pr

                def do_pv(kind, ss, hi, pr, p=p, ctx2=ctx2):
                    h = 2 * p + hi
                    g = 0 if h < 4 else (1 if h < 8 else 2)
                    for j in range(2):
                        t = 2 * ss + j
                        if kind == "bg":
                            vlhs = vbg_t[p][:, t, 65 * hi : 65 * (hi + 1)]
                        else:
                            ensure(("v", g, t))
                            vlhs = v2[:, t, h, :]
                        first = kind == "bg" and ss == 0 and j == 0
                        last = kind == "self" and ss == 15 and j == 1
                        nc.tensor.matmul(
                            ctx2[:, hi, :],
                            lhsT=vlhs,
                            rhs=pr[:, j, :],
                            start=first,
                            stop=last,
                        )

                steps = [("bg", ss, hi) for ss in range(16) for hi in range(2)]
                steps += [("self", ss, hi) for ss in range(16) for hi in range(2)]
                for si, (kind, ss, hi) in enumerate(steps):
                    if si == 8 and p > 0:
                        work.extend(o_items(p - 1))
                    if kind == "bg" and ss == 0:
                        ensure(("q", 2 * p + hi))
                    if kind == "self" and ss == 0:
                        for f in range(4):
                            ensure(("k", p, hi, f))
                    n_dr = 0 if (p == 0 and si < 8) else 2
                    pr = superstep(kind, ss, hi)
                    drain(n_dr // 2)
                    while pend:
                        do_pv(*pend.popleft())
                    drain(n_dr - n_dr // 2)
                    pend.append((kind, ss, hi, pr))
                while pend:
                    do_pv(*pend.popleft())

                # free ctx PSUM immediately; denominators run as items
                if p >= 2:
                    # ctxU reuses pair p-2's buffer; o-items of p-2 (queued
                    # into pair p-1's self phase) must have consumed it
                    ensure(("o", p - 2, Q // 128 - 1, 1))
                ctxU = fin_pool.tile(
                    [D + 1, 2, Q], F16, tag="ctxU", name=f"cu{p}", bufs=2
                )
                sbuf_live["ctxU", p] = ctxU
                nc.vector.tensor_copy(out=ctxU, in_=ctx2)
                work.extend(n_items(p))
                if p == n_pair - 1:
                    work.extend(o_items(p))

            drain(len(work))
    return nc


def split_waits(nc, limit=1):
    """Hoist excess sync waits onto standalone EventSemaphore instructions."""
    cnt = 0
    for f in nc.m.functions:
        for bb in f.blocks:
            fixed = []
            for inst in bb.instructions:
                si = inst.sync_info
                if si is not None and len(si.on_wait) > limit:
                    waits = list(si.on_wait)
                    extra, keep = waits[:-limit], waits[-limit:]
                    for w in extra:
                        cnt += 1
                        ev = mybir.InstEventSemaphore(
                            name=f"I-waitsplit-{cnt}", ins=[], outs=[]
                        )
                        ev.engine = inst.engine
                        ev.sync_info = mybir.SyncInfo(on_wait=[w], on_update=[])
                        nc.register_instruction(ev)
                        fixed.append(ev)
                    si.on_wait = keep
                fixed.append(inst)
            bb.instructions[:] = fixed
    return cnt


def build_bass():
    nc = bass.Bass()
    emit(nc)
    split_waits(nc)
    return nc


def make_in_maps(hidden_states, K_bg, V_bg, Wq, Wk, Wv, Wo, bo):
    f16 = np.float16
    hidden = np.asarray(hidden_states, np.float32)[0]  # [L, C]
    hT5 = np.ascontiguousarray(hidden.T.reshape(NCC, 128, L)).astype(f16)

    def chunk_w(W):
        return np.ascontiguousarray(
            np.asarray(W, np.float32).reshape(NCC, 128, C)
        ).astype(f16)

    wq5, wk5, wv5 = chunk_w(Wq), chunk_w(Wk), chunk_w(Wv)

    WoB = np.zeros((H, D + 1, C), np.float32)
    WoB[:, :D, :] = np.asarray(Wo, np.float32).reshape(H, D, C)
    WoB[0, D, :] = np.asarray(bo, np.float32)

    # bg K: [H, 128, L/2], alpha folded, even key tiles on rows 0:64
    KbgT = np.asarray(K_bg, np.float32).transpose(0, 2, 1) * ALPHA  # [H, D, L]
    kv = KbgT.reshape(H, D, NKT, 128)
    kbgS = np.empty((H, 128, L // 2), np.float32)
    kbgS[:, 0:D, :] = kv[:, :, 0::2, :].reshape(H, D, L // 2)
    kbgS[:, D:128, :] = kv[:, :, 1::2, :].reshape(H, D, L // 2)
    kbgS = np.ascontiguousarray(kbgS).astype(f16)

    # bg V: [n_pair, 128, NKT*130], alpha folded, ones baked at cols 64/129
    Vb = (np.asarray(V_bg, np.float32) * ALPHA).reshape(H, NKT, 128, D)
    vbgS = np.ones((H // 2, 128, NKT, 130), np.float32)
    for hi in range(2):
        vbgS[:, :, :, 65 * hi : 65 * hi + D] = Vb[hi::2].transpose(0, 2, 1, 3)
    vbgS = np.ascontiguousarray(vbgS.reshape(H // 2, 128, NKT * 130)).astype(f16)

    common = {
        "hT": hT5,
        "wq": wq5,
        "wk": wk5,
        "wv": wv5,
        "wob": WoB.astype(f16),
        "kbgS": kbgS,
        "vbgS": vbgS,
    }
    return [
        dict(common, hq=np.ascontiguousarray(hT5[:, :, Q * c : Q * (c + 1)]))
        for c in range(N_CORES)
    ]


_NC_CACHE = {}


def kernel(hidden_states, K_bg, V_bg, Wq, Wk, Wv, Wo, bo):
    if "nc" not in _NC_CACHE:
        _NC_CACHE["nc"] = build_bass()
    nc = _NC_CACHE["nc"]
    in_maps = make_in_maps(hidden_states, K_bg, V_bg, Wq, Wk, Wv, Wo, bo)
    from concourse import bass2jax

    results = bass2jax.run_bass_via_pjrt(nc, in_maps, n_cores=N_CORES)
    out = np.concatenate(
        [np.asarray(results[c]["out"], np.float32) for c in range(N_CORES)], axis=0
    )
    return out.reshape(B, L, C)
